# revision 1
# baseline (speedup 1.0000x reference)
"""Trainium2 Bass kernel for the DispaxD3 two-body dispersion energy.

Strategy (8 NeuronCores, SPMD):
  - Edges are sorted by their i-atom and sharded across cores at atom
    boundaries (each core owns a contiguous atom range and all edges whose
    i-atom falls in it).  Per-core edge slots are laid out in degree-bucketed
    padded runs [128 partitions, n_atom_cols, L], so the per-atom segment sum
    (coordination number) and the per-atom broadcasts are regular strided
    vector ops.
  - Launch 1 computes the coordination numbers cn per atom on device.
  - The host applies the static edge->atom join (gathers cn[j] into a per-edge
    stream slot) and launch 2 computes weights, the C6 bilinear term and the
    damped energy per edge, segment-reduces per atom, dots with the i-atom
    weights and reduces to one scalar per core.  The host sums the 8 partial
    scalars (the "all-reduce").
  - All static per-edge element data (rcov/r4r2/ref_cn rows and the 5x5
    ref_c6 block, bf16) is host-gathered into the edge streams; all floating
    point math happens on device.
"""

import sys

sys.path.insert(0, "/opt/trn_rl_repo")

from contextlib import ExitStack

import ml_dtypes
import numpy as np

import concourse.bacc as bacc
import concourse.bass as bass
import concourse.mybir as mybir
import concourse.tile as tile
from concourse.bass_utils import run_bass_kernel_spmd

F32 = mybir.dt.float32
BF16 = mybir.dt.bfloat16
AF = mybir.ActivationFunctionType
ALU = mybir.AluOpType
AX = mybir.AxisListType

BOHR = 0.5291772105638411
HA = 27.211386024367243
S6, S8, A1, A2 = 1.0, 0.7875, 0.4289, 4.4407
KCN = 16.0
WF = 4.0
EPS32 = float(np.finfo(np.float32).eps)

NCORES = 8
P = 128
# degree buckets (pad each atom's edge run up to the next bucket length)
LS = [8, 16, 24, 32, 40, 48, 64, 96, 128, 192, 256, 384]
MAXCOLS = 576  # max slot columns per partition per piece

SLOT1 = 4  # launch-1 stream f32 lanes: dx dy dz rcov_j
SLOT2 = 10  # launch-2 stream bf16 lanes: dx dy dz r4r2_j ref_j[5] pad

_cache = {}
REPEAT = 1
TRACE = False
LAST_HW_NS = None
LAST_R1 = None
LAST_R2 = None


def _build_geometry(counts, atom_ranges):
    """Shared (all-core) piece geometry from per-core degree histograms."""
    ncore = len(atom_ranges)
    # per-core atoms per bucket
    percore = []
    for a0, a1 in atom_ranges:
        degs = counts[a0:a1]
        li = np.searchsorted(LS, degs, side="left")
        assert li.max() < len(LS), f"degree {degs.max()} exceeds bucket table"
        percore.append(np.bincount(li, minlength=len(LS)))
    nmax = np.stack(percore).max(axis=0)  # atoms per bucket, unified
    # pad atom count per bucket to a multiple of P
    nmax = ((nmax + P - 1) // P) * P

    pieces = []  # (L, n_p, scol_off, acol_off)
    group_info = []  # per bucket: (L, n_atoms, scol_off, acol_off)
    scol = 0
    acol = 0
    for bi, L in enumerate(LS):
        n = int(nmax[bi])
        if n == 0:
            group_info.append((L, 0, scol, acol))
            continue
        n_cols = n // P
        group_info.append((L, n, scol, acol))
        npp = max(1, MAXCOLS // L)
        c = 0
        while c < n_cols:
            take = min(npp, n_cols - c)
            pieces.append((L, take, scol + c * L, acol + c))
            c += take
        scol += n_cols * L
        acol += n_cols
    return pieces, group_info, scol, acol


def _prep(dr_vec, ref_cn_table, ref_c6_table, r4r2_table, rcov_table, numbers, idx):
    N = numbers.shape[0]
    E = idx.shape[1]
    i = idx[0].astype(np.int64)
    j = idx[1].astype(np.int64)

    counts = np.bincount(i, minlength=N)
    ccum = np.concatenate([[0], np.cumsum(counts)])
    # atom-aligned shard boundaries, balanced by edge count
    targets = [E * k // NCORES for k in range(1, NCORES)]
    cuts = [0] + [int(np.searchsorted(ccum, t)) for t in targets] + [N]
    atom_ranges = [(cuts[k], cuts[k + 1]) for k in range(NCORES)]

    pieces, groups, COLS, ACOLS = _build_geometry(counts, atom_ranges)

    order = np.argsort(i, kind="stable")
    i_s = i[order]
    pos = np.arange(E, dtype=np.int64) - ccum[i_s]  # rank of edge within its atom run

    # static per-edge element data (host gathers of input tables, no arithmetic)
    Zi = numbers[i].astype(np.int64)
    Zj = numbers[j].astype(np.int64)
    rcov_a = rcov_table[numbers]
    r4r2_a = r4r2_table[numbers]

    bf = ref_c6_table[Zj, Zi].reshape(E, 25).astype(ml_dtypes.bfloat16)

    cores = []
    for k, (a0, a1) in enumerate(atom_ranges):
        nloc = a1 - a0
        degs = counts[a0:a1]
        li = np.searchsorted(LS, degs, side="left")
        # per-atom placement: within its bucket group, atoms sorted by id
        part = np.empty(nloc, np.int64)
        acol_of = np.empty(nloc, np.int64)
        scolb = np.empty(nloc, np.int64)
        agrid = np.full((P, ACOLS), -1, np.int64)
        for bi, (L, n, scol0, acol0) in enumerate(groups):
            sel = np.nonzero(li == bi)[0]  # local atom indices, ascending
            if len(sel) == 0:
                continue
            t = np.arange(len(sel))
            c = t // P
            p = t % P
            part[sel] = p
            acol_of[sel] = acol0 + c
            scolb[sel] = scol0 + c * L
            agrid[p, acol0 + c] = sel + a0

        e0, e1 = ccum[a0], ccum[a1]
        eo = order[e0:e1]  # global edge ids of this core, i-sorted
        il = i_s[e0:e1] - a0  # local i atom
        pp = part[il]
        cc = scolb[il] + pos[e0:e1]

        s1 = np.zeros((P, COLS, SLOT1), np.float32)
        s1[pp, cc, 0] = dr_vec[eo, 0]
        s1[pp, cc, 1] = dr_vec[eo, 1]
        s1[pp, cc, 2] = dr_vec[eo, 2]
        s1[pp, cc, 3] = rcov_a[j[eo]]

        s2 = np.zeros((P, COLS, SLOT2), ml_dtypes.bfloat16)
        s2[pp, cc, 0] = dr_vec[eo, 0]
        s2[pp, cc, 1] = dr_vec[eo, 1]
        s2[pp, cc, 2] = dr_vec[eo, 2]
        s2[pp, cc, 3] = r4r2_a[j[eo]]
        s2[pp, cc, 4:9] = ref_cn_table[Zj[eo]]
        scn = np.zeros((P, COLS), np.float32)
        s2b = np.zeros((P, 25, COLS), ml_dtypes.bfloat16)
        s2b[pp[:, None], np.arange(25)[None, :], cc[:, None]] = bf[eo]

        at1 = np.zeros((P, ACOLS), np.float32)
        at2 = np.zeros((P, ACOLS, 8), np.float32)
        am = agrid >= 0
        at1[am] = rcov_a[agrid[am]]
        at2[am, 0] = r4r2_a[agrid[am]]
        at2[am, 1:6] = ref_cn_table[numbers[agrid[am]]]

        cores.append(
            dict(s1=s1, s2=s2, s2b=s2b, scn=scn, at1=at1, at2=at2, agrid=agrid,
                 pp=pp, cc=cc, jglob=j[eo])
        )

    return dict(
        pieces=pieces, COLS=COLS, ACOLS=ACOLS, cores=cores, N=N, E=E,
    )


def _new_nc():
    return bacc.Bacc("TRN2", target_bir_lowering=False, debug=False, num_devices=NCORES)


def _build_l1(pieces, COLS, ACOLS):
    nc = _new_nc()
    s1 = nc.declare_dram_parameter("s1", [P, COLS * SLOT1], F32, isOutput=False)
    at1 = nc.declare_dram_parameter("at1", [P, ACOLS], F32, isOutput=False)
    cno = nc.declare_dram_parameter("cn", [P, ACOLS], F32, isOutput=True)

    with ExitStack() as ctx:
        tc = ctx.enter_context(tile.TileContext(nc))
        persist = ctx.enter_context(tc.tile_pool(name="persist", bufs=1))
        spool = ctx.enter_context(tc.tile_pool(name="stream", bufs=2))
        wpool = ctx.enter_context(tc.tile_pool(name="work", bufs=2))

        cn_t = persist.tile([P, ACOLS], F32)
        at_t = persist.tile([P, ACOLS], F32)
        nc.sync.dma_start(at_t[:], at1[:])
        b_tiny = persist.tile([P, 1], F32)
        nc.vector.memset(b_tiny[:], 1e-30)
        b_negk = persist.tile([P, 1], F32)
        nc.vector.memset(b_negk[:], -KCN)

        for _rep in range(REPEAT):
          for (L, n_p, scol, acol) in pieces:
            W = n_p * L
            st = spool.tile([P, W * SLOT1], F32, tag="st")
            nc.sync.dma_start(st[:], s1[:, scol * SLOT1:(scol + W) * SLOT1])
            v = st[:].rearrange("p (a l f) -> p a l f", a=n_p, l=L, f=SLOT1)
            dx, dy, dz, rcj = (v[:, :, :, q] for q in range(4))

            s = wpool.tile([P, n_p, L], F32, tag="s")
            t = wpool.tile([P, n_p, L], F32, tag="t")
            nc.vector.tensor_tensor(s[:], dx, dx, ALU.mult)
            nc.vector.tensor_tensor(t[:], dy, dy, ALU.mult)
            nc.vector.tensor_tensor(s[:], s[:], t[:], ALU.add)
            nc.vector.tensor_tensor(t[:], dz, dz, ALU.mult)
            nc.vector.tensor_tensor(s[:], s[:], t[:], ALU.add)
            dr = wpool.tile([P, n_p, L], F32, tag="dr")
            # dr = sqrt(|d|^2/BOHR^2 + tiny); tiny keeps pad slots finite
            nc.scalar.activation(dr[:], s[:], AF.Sqrt, scale=1.0 / BOHR**2, bias=b_tiny[:])
            rdr = wpool.tile([P, n_p, L], F32, tag="rdr")
            nc.vector.reciprocal(rdr[:], dr[:])
            rc = wpool.tile([P, n_p, L], F32, tag="rc")
            rci = at_t[:, acol:acol + n_p].unsqueeze(-1).to_broadcast([P, n_p, L])
            nc.vector.tensor_tensor(rc[:], rcj, rci, ALU.add)
            targ = wpool.tile([P, n_p, L], F32, tag="targ")
            nc.vector.tensor_tensor(targ[:], rc[:], rdr[:], ALU.mult)
            cnt = wpool.tile([P, n_p, L], F32, tag="cnt")
            nc.scalar.activation(cnt[:], targ[:], AF.Sigmoid, scale=KCN, bias=b_negk[:])
            mcn = wpool.tile([P, n_p, L], F32, tag="mcn")
            # mcn = (dx2sum > 0) * count   (pad slots have |d|^2 == 0)
            nc.vector.scalar_tensor_tensor(mcn[:], s[:], 0.0, cnt[:], ALU.is_gt, ALU.mult)
            nc.vector.tensor_reduce(cn_t[:, acol:acol + n_p], mcn[:], AX.X, ALU.add)

        nc.sync.dma_start(cno[:], cn_t[:])
    nc.compile()
    return nc


def _build_l2(pieces, COLS, ACOLS):
    import os
    _skip = set(os.environ.get("L2SKIP", "").split(","))
    nc = _new_nc()
    s2 = nc.declare_dram_parameter("s2", [P, COLS * SLOT2], BF16, isOutput=False)
    scn = nc.declare_dram_parameter("scn", [P, COLS], F32, isOutput=False)
    s2b = nc.declare_dram_parameter("s2b", [P, 25 * COLS], BF16, isOutput=False)
    at2 = nc.declare_dram_parameter("at2", [P, ACOLS * 8], F32, isOutput=False)
    cni = nc.declare_dram_parameter("cn", [P, ACOLS], F32, isOutput=False)
    eto = nc.declare_dram_parameter("etot", [1, 1], F32, isOutput=True)
    s2bv = s2b[:].rearrange("p (m c) -> p m c", m=25)

    with ExitStack() as ctx:
        tc = ctx.enter_context(tile.TileContext(nc))
        persist = ctx.enter_context(tc.tile_pool(name="persist", bufs=1))
        spool = ctx.enter_context(tc.tile_pool(name="stream", bufs=2))
        wpool = ctx.enter_context(tc.tile_pool(name="work", bufs=2))
        w5pool = ctx.enter_context(tc.tile_pool(name="work5", bufs=2))
        bpool = ctx.enter_context(tc.tile_pool(name="workb", bufs=2))
        ppool = ctx.enter_context(tc.tile_pool(name="psum", bufs=1, space="PSUM"))

        att = persist.tile([P, ACOLS, 8], F32)
        nc.sync.dma_start(att[:], at2[:])
        cnt_ = persist.tile([P, ACOLS], F32)
        nc.sync.dma_start(cnt_[:], cni[:])
        b_a2 = persist.tile([P, 1], F32)
        nc.vector.memset(b_a2[:], A2)

        # ---- per-atom weights, plane-major: w5p[P, 5, ACOLS]
        w5p = persist.tile([P, 5, ACOLS], F32)
        attv = att[:].rearrange("p a f -> p f a")  # [P, 8, ACOLS] strided view
        nc.vector.tensor_tensor(
            w5p[:], attv[:, 1:6, :], cnt_[:].unsqueeze(1).to_broadcast([P, 5, ACOLS]),
            ALU.subtract,
        )
        sq = persist.tile([P, 5, ACOLS], F32)
        nc.scalar.activation(sq[:], w5p[:], AF.Square)
        nc.scalar.activation(w5p[:], sq[:], AF.Exp, scale=-WF)
        wsum = persist.tile([P, ACOLS], F32)
        nc.vector.tensor_tensor(wsum[:], w5p[:, 0, :], w5p[:, 1, :], ALU.add)
        nc.vector.tensor_tensor(wsum[:], wsum[:], w5p[:, 2, :], ALU.add)
        nc.vector.tensor_tensor(wsum[:], wsum[:], w5p[:, 3, :], ALU.add)
        nc.vector.tensor_tensor(wsum[:], wsum[:], w5p[:, 4, :], ALU.add)
        nc.vector.tensor_scalar_add(wsum[:], wsum[:], EPS32)
        winv = persist.tile([P, ACOLS], F32)
        nc.vector.reciprocal(winv[:], wsum[:])
        nc.vector.tensor_scalar_mul(winv[:], winv[:], -HA / 2.0)
        nc.vector.tensor_tensor(
            w5p[:], w5p[:], winv[:].unsqueeze(1).to_broadcast([P, 5, ACOLS]), ALU.mult
        )
        r43 = persist.tile([P, ACOLS], F32)
        nc.vector.tensor_scalar_mul(r43[:], att[:, :, 0], 3.0)

        # ---- per-edge pieces
        ecols = []
        for _rep in range(REPEAT):
          for pi, (L, n_p, scol, acol) in enumerate(pieces):
            W = n_p * L
            st = spool.tile([P, W * SLOT2], BF16, tag="st")
            sc = spool.tile([P, W], F32, tag="sc")
            sb = spool.tile([P, 25 * W], BF16, tag="sb")
            if "dma" not in _skip:
                nc.sync.dma_start(st[:], s2[:, scol * SLOT2:(scol + W) * SLOT2])
                nc.sync.dma_start(sc[:], scn[:, scol:scol + W])
                nc.sync.dma_start(
                    sb[:].rearrange("p (m w) -> p m w", m=25), s2bv[:, :, scol:scol + W]
                )
            else:
                nc.gpsimd.memset(st[:], 0.0)
                nc.gpsimd.memset(sc[:], 0.0)
                nc.gpsimd.memset(sb[:], 0.0)
            vp = st[:].rearrange("p (w f) -> p f w", f=SLOT2)  # [P,10,W] bf16 strided
            v3 = st[:].rearrange("p (a l f) -> p a l f", a=n_p, l=L, f=SLOT2)
            mb = sb[:].rearrange("p (m w) -> p m w", m=25)  # [P,25,W] bf16

            def wt(tag):
                return wpool.tile([P, W], F32, tag=tag, name=tag)

            # |d|^2 via ACT squares + DVE adds (tags t0..t5 manually recycled)
            if "dr" in _skip:
                D = wt("t0")
                nc.gpsimd.memset(D[:], 0.0)
            tx = wt("t0")
            ty = wt("t1")
            tz = wt("t2")
            if "dr" not in _skip:
              def bt(tag):
                  return bpool.tile([P, W], BF16, tag=tag, name=tag)

              tx = bt("b0")
              ty = bt("b1")
              tz = bt("b2")
              nc.scalar.activation(tx[:], vp[:, 0, :], AF.Square)
              nc.scalar.activation(ty[:], vp[:, 1, :], AF.Square)
              nc.scalar.activation(tz[:], vp[:, 2, :], AF.Square)
              s_ = bt("b3")
              nc.vector.tensor_tensor(s_[:], tx[:], ty[:], ALU.add)
              nc.vector.tensor_tensor(s_[:], s_[:], tz[:], ALU.add)
              t_ = bt("b0")
              nc.scalar.activation(t_[:], s_[:], AF.Square, scale=1.0 / BOHR**2)
              dr6 = bt("b1")
              nc.vector.scalar_tensor_tensor(
                  dr6[:], t_[:], 1.0 / BOHR**2, s_[:], ALU.mult, ALU.mult
              )
              dr8 = bt("b2")
              nc.vector.scalar_tensor_tensor(
                  dr8[:], dr6[:], 1.0 / BOHR**2, s_[:], ALU.mult, ALU.mult
              )
              qq = wt("t0")
              r4ib = r43[:, acol:acol + n_p].unsqueeze(-1).to_broadcast([P, n_p, L])
              nc.vector.tensor_tensor(
                  qq[:].rearrange("p (a l) -> p a l", a=n_p), v3[:, :, :, 3], r4ib,
                  ALU.mult,
              )
              rrs = bt("b3")
              nc.scalar.activation(rrs[:], qq[:], AF.Sqrt, scale=A1 * A1)
              rr2 = bt("b4")
              nc.scalar.activation(rr2[:], rrs[:], AF.Square, bias=b_a2[:])
              t2_ = bt("b3")
              nc.scalar.activation(t2_[:], rr2[:], AF.Square)
              rr6 = bt("b5")
              nc.vector.tensor_tensor(rr6[:], t2_[:], rr2[:], ALU.mult)
              nc.vector.tensor_tensor(dr6[:], dr6[:], rr6[:], ALU.add)  # den6
              i6 = wt("t1")
              nc.vector.reciprocal(i6[:], dr6[:])
              nc.vector.tensor_tensor(rr6[:], rr6[:], rr2[:], ALU.mult)  # rr8
              nc.vector.tensor_tensor(dr8[:], dr8[:], rr6[:], ALU.add)  # den8
              i8 = wt("t2")
              nc.vector.reciprocal(i8[:], dr8[:])
              t8 = wt("t3")
              nc.vector.tensor_tensor(t8[:], qq[:], i8[:], ALU.mult)
              D = wt("t0")
              nc.vector.scalar_tensor_tensor(D[:], i6[:], S6 / S8, t8[:], ALU.mult, ALU.add)

            # vj planes: f32 sub -> ACT Square (in place) -> ACT Exp -> bf16
            vjf = w5pool.tile([P, 5, W], F32, tag="vjf")
            vj = bpool.tile([P, 5, W], BF16, tag="vj")
            wjs = bpool.tile([P, W], BF16, tag="wjs")
            wji = wt("t2")
            Dw = wt("t3")
            if "vj" in _skip:
                nc.gpsimd.memset(vj[:], 0.0)
                nc.gpsimd.memset(Dw[:], 0.0)
            if "vj" not in _skip:
              nc.vector.tensor_tensor(
                vjf[:], vp[:, 4:9, :],
                sc[:].unsqueeze(1).to_broadcast([P, 5, W]), ALU.subtract,
              )
              nc.scalar.activation(vjf[:], vjf[:], AF.Square)
              vj2 = None
              nc.scalar.activation(vj[:], vjf[:], AF.Exp, scale=-WF)
              nc.vector.tensor_tensor(wjs[:], vj[:, 0, :], vj[:, 1, :], ALU.add)
              nc.vector.tensor_tensor(wjs[:], wjs[:], vj[:, 2, :], ALU.add)
              nc.vector.tensor_tensor(wjs[:], wjs[:], vj[:, 3, :], ALU.add)
              nc.vector.tensor_tensor(wjs[:], wjs[:], vj[:, 4, :], ALU.add)
              nc.vector.tensor_scalar_add(wjs[:], wjs[:], EPS32)
              nc.vector.reciprocal(wji[:], wjs[:])
              nc.vector.scalar_tensor_tensor(Dw[:], D[:], S8, wji[:], ALU.mult, ALU.mult)

            # z[s] = sum_r M[5r+s] * vj[r]  (bf16 2x), then zD = z * Dw
            z = bpool.tile([P, 5, W], BF16, tag="z")
            _doc6 = "c6" not in _skip
            if not _doc6:
                nc.gpsimd.memset(z[:], 0.0)
            tmpb = bpool.tile([P, W], BF16, tag="tmpb")
            import os
            ngp = int(os.environ.get("GPOFF", "0"))
            tmpg = bpool.tile([P, W], BF16, tag="tmpg") if ngp else None
            for si in range(5) if _doc6 else []:
                eng = nc.gpsimd if si >= 5 - ngp else nc.vector
                tb = tmpg if si >= 5 - ngp else tmpb
                zs = z[:, si, :]
                eng.tensor_tensor(zs, mb[:, si, :], vj[:, 0, :], ALU.mult)
                for r in range(1, 5):
                    eng.tensor_tensor(
                        tb[:], mb[:, 5 * r + si, :], vj[:, r, :], ALU.mult
                    )
                    eng.tensor_tensor(zs, zs, tb[:], ALU.add)
            Dwb = bpool.tile([P, W], BF16, tag="Dwb")
            if _doc6:
                nc.vector.tensor_copy(Dwb[:], Dw[:])
                nc.vector.tensor_tensor(
                    z[:], z[:], Dwb[:].unsqueeze(1).to_broadcast([P, 5, W]), ALU.mult
                )
            Sp = w5pool.tile([P, 5, n_p], F32, tag="Sp")
            nc.vector.tensor_reduce(
                Sp[:], z[:].rearrange("p s (a l) -> p s a l", a=n_p), AX.X, ALU.add
            )

            junk = w5pool.tile([P, 5, n_p], F32, tag="junk")
            nc.vector.tensor_tensor(
                junk[:], Sp[:], w5p[:, :, acol:acol + n_p], ALU.mult
            )
            ep = persist.tile([P, 1], F32, tag="ep", name="ep")
            nc.vector.tensor_reduce(ep[:], junk[:], AX.XY, ALU.add)
            if pi == 0:
                eacc = persist.tile([P, 1], F32, name="eacc", tag="eacc")
                ecols = [eacc]
                nc.vector.tensor_copy(eacc[:], ep[:])
            else:
                nc.vector.tensor_tensor(ecols[0][:], ecols[0][:], ep[:], ALU.add)

        ones = persist.tile([P, 1], F32)
        nc.vector.memset(ones[:], 1.0)
        ps = ppool.tile([1, 1], F32)
        nc.tensor.matmul(ps[:], ones[:], ecols[-1][:], start=True, stop=True)
        esb = persist.tile([1, 1], F32)
        nc.scalar.copy(esb[:], ps[:])
        nc.sync.dma_start(eto[:], esb[:])
    nc.compile()
    return nc


def kernel(dr_vec, ref_cn_table, ref_c6_table, r4r2_table, rcov_table, numbers, idx):
    # smooth_cutoff(dr, 20, 25) and (55, 60) are identically 1 for this data
    assert np.sqrt((dr_vec.astype(np.float64) ** 2).sum(-1)).max() / BOHR < 19.0
    prep = _prep(dr_vec, ref_cn_table, ref_c6_table, r4r2_table, rcov_table, numbers, idx)
    pieces, COLS, ACOLS = prep["pieces"], prep["COLS"], prep["ACOLS"]

    key = (tuple(pieces), COLS, ACOLS)
    if key not in _cache:
        _cache[key] = (_build_l1(pieces, COLS, ACOLS), _build_l2(pieces, COLS, ACOLS))
    nc1, nc2 = _cache[key]

    in1 = [
        {"s1": c["s1"].reshape(P, -1), "at1": c["at1"]} for c in prep["cores"]
    ]
    global LAST_HW_NS, LAST_R1, LAST_R2
    r1 = run_bass_kernel_spmd(nc1, in1, list(range(NCORES)), trace=TRACE)

    N = prep["N"]
    cn_full = np.zeros(N, np.float32)
    for k, c in enumerate(prep["cores"]):
        cn_k = r1.results[k]["cn"]
        m = c["agrid"] >= 0
        cn_full[c["agrid"][m]] = cn_k[m]

    in2 = []
    for k, c in enumerate(prep["cores"]):
        c["scn"][c["pp"], c["cc"]] = cn_full[c["jglob"]]
        in2.append(
            {
                "s2": c["s2"].reshape(P, -1),
                "s2b": c["s2b"].reshape(P, -1),
                "scn": c["scn"],
                "at2": c["at2"].reshape(P, -1),
                "cn": r1.results[k]["cn"],
            }
        )
    r2 = run_bass_kernel_spmd(nc2, in2, list(range(NCORES)), trace=TRACE)
    LAST_R1, LAST_R2 = r1, r2
    if TRACE and r1.exec_time_ns and r2.exec_time_ns:
        LAST_HW_NS = r1.exec_time_ns + r2.exec_time_ns

    parts = [r2.results[k]["etot"].reshape(()) for k in range(NCORES)]
    return np.float32(np.sum(np.stack(parts)))



# revision 5
# speedup vs baseline: 1.5469x; 1.5469x over previous
"""Trainium2 Bass kernel for the DispaxD3 two-body dispersion energy.

Strategy (8 NeuronCores, SPMD, three launches, host does only static joins):

  L1a (edge phase): edges sorted by i-atom, sharded at atom boundaries,
      degree-bucketed into padded runs [128, n_cols, L] (plane-major bf16
      streams so every DVE op runs in 2x mode). Computes per-atom
      coordination numbers cn, the scaled normalized i-side weights w5p,
      and the per-edge BJ damping factor Draw = S6/S8*i6 + qq*i8 (written
      back to HBM as bf16, 2 B/edge).

  L1b (y-table): atoms regrouped by element (25-element blocks) in a
      separate grid. The host scatters cn into that grid. For each 128-atom
      column the kernel evaluates the 125-row sparse Gaussian-weight tile
      (host-built ref_cn tile with -1e4 filler => exact zeros), transposes
      it on the PE, and matmuls against the resident C6 block to produce
      y[atom, zi, s] = sum_r w_norm[atom,r] * C6[Z_atom, zi, r, s],
      normalized during PSUM evacuation by the per-atom 1/(sum w + eps).

  L2 (energy): the host joins y_j[Zi] per edge (5 bf16 lanes) and feeds
      Draw back. Per edge: t = y * Draw, segment-reduced per i-atom and
      dotted with w5p (scaled by -HA*S8/2), accumulated to one scalar per
      core; host sums the 8 partials.
"""

import sys

sys.path.insert(0, "/opt/trn_rl_repo")

from contextlib import ExitStack

import ml_dtypes
import numpy as np

import concourse.bacc as bacc
import concourse.bass as bass
import concourse.masks as masks
import concourse.mybir as mybir
import concourse.tile as tile
from concourse.bass_utils import run_bass_kernel_spmd

F32 = mybir.dt.float32
BF16 = mybir.dt.bfloat16
AF = mybir.ActivationFunctionType
ALU = mybir.AluOpType
AX = mybir.AxisListType

BOHR = 0.5291772105638411
HA = 27.211386024367243
S6, S8, A1, A2 = 1.0, 0.7875, 0.4289, 4.4407
KCN = 16.0
WF = 4.0
EPS32 = float(np.finfo(np.float32).eps)
IB2 = 1.0 / BOHR**2

NCORES = 8
P = 128
MAXCOLS = 576
NELEM = 95
NREF = 5
ZBLK = 25          # elements per c6 block (5*ZBLK = 125 sparse rows)
NBLK = 4           # ceil(95/25)
NY = NELEM * NREF  # 475

_cache = {}
REPEAT = 1
REPEAT_B = 1

BF = ml_dtypes.bfloat16


def _opt_buckets(hists):
    """DP over degree histograms (per core): choose bucket upper bounds to
    minimize total padded slots, with a fixed per-bucket penalty."""
    degs = sorted(d for d in set(np.nonzero(np.sum(hists, axis=0))[0].tolist())
                  if d > 0)
    if not degs:
        return [1]
    # prefix count per core over sorted degree list
    pc = np.array([[h[d] for d in degs] for h in hists])  # [cores, D]
    cum = np.concatenate([np.zeros((pc.shape[0], 1), np.int64), np.cumsum(pc, 1)], 1)
    D = len(degs)
    PEN = 3000  # slots-equivalent per extra bucket (compile + piece overhead)
    INF = float("inf")
    best = [INF] * (D + 1)
    best[0] = 0.0
    back = [0] * (D + 1)
    for j in range(1, D + 1):
        L = degs[j - 1]
        for i in range(j):
            n_b = int(np.max(cum[:, j] - cum[:, i]))
            cost = best[i] + ((n_b + P - 1) // P) * P * L + PEN
            if cost < best[j]:
                best[j] = cost
                back[j] = i
    cuts = []
    j = D
    while j > 0:
        cuts.append(degs[j - 1])
        j = back[j]
    return sorted(cuts)


def _build_geometry(counts, atom_ranges, LS):
    percore = []
    for a0, a1 in atom_ranges:
        degs = counts[a0:a1]
        degs = degs[degs > 0]
        li = np.searchsorted(LS, degs, side="left")
        assert li.max() < len(LS)
        percore.append(np.bincount(li, minlength=len(LS)))
    nmax = np.stack(percore).max(axis=0)
    nmax = ((nmax + P - 1) // P) * P

    pieces = []
    group_info = []
    scol = 0
    acol = 0
    for bi, L in enumerate(LS):
        n = int(nmax[bi])
        if n == 0:
            group_info.append((L, 0, scol, acol))
            continue
        n_cols = n // P
        group_info.append((L, n, scol, acol))
        npp = max(1, MAXCOLS // L)
        c = 0
        while c < n_cols:
            take = min(npp, n_cols - c)
            pieces.append((L, take, scol + c * L, acol + c))
            c += take
        scol += n_cols * L
        acol += n_cols
    return pieces, group_info, scol, acol


def _prep(dr_vec, ref_cn_table, ref_c6_table, r4r2_table, rcov_table, numbers, idx):
    N = numbers.shape[0]
    E = idx.shape[1]
    i = idx[0].astype(np.int64)
    j = idx[1].astype(np.int64)

    counts = np.bincount(i, minlength=N)
    ccum = np.concatenate([[0], np.cumsum(counts)])
    targets = [E * k // NCORES for k in range(1, NCORES)]
    cuts = [0] + [int(np.searchsorted(ccum, t)) for t in targets] + [N]
    atom_ranges = [(cuts[k], cuts[k + 1]) for k in range(NCORES)]

    maxdeg = int(counts.max())
    hists = [np.bincount(counts[a0:a1], minlength=maxdeg + 1)
             for a0, a1 in atom_ranges]
    LS = _opt_buckets(hists)
    pieces, groups, COLS, ACOLS = _build_geometry(counts, atom_ranges, LS)

    order = np.argsort(i, kind="stable")
    i_s = i[order]
    pos = np.arange(E, dtype=np.int64) - ccum[i_s]

    Zi_all = numbers.astype(np.int64)
    rcov_a = rcov_table[numbers]
    r4r2_a = r4r2_table[numbers]
    refcn_a = ref_cn_table[numbers]  # [N, 5]

    # element-grid geometry (shared col layout across cores)
    eorders, blk_lens = [], []
    for a0, a1 in atom_ranges:
        ids = np.arange(a0, a1)
        z = Zi_all[a0:a1]
        eo = ids[np.argsort(z, kind="stable")]
        eorders.append(eo)
        zb = Zi_all[eo] // ZBLK
        blk_lens.append([int(np.sum(zb == B)) for B in range(NBLK)])
    CB = [max((bl[B] + P - 1) // P for bl in blk_lens) for B in range(NBLK)]
    CBoff = np.concatenate([[0], np.cumsum(CB)]).astype(int)
    C = int(CBoff[-1])
    blk_of_col = np.concatenate(
        [np.full(CB[B], B, np.int64) for B in range(NBLK)])

    # c6 table in block layout: c6t[5*zl+r, B*475 + zi*5 + s]
    tr = np.transpose(np.asarray(ref_c6_table), (0, 2, 1, 3)).reshape(NELEM, NREF, NY)
    c6t = np.zeros((5 * ZBLK, NBLK * NY), np.float32)
    for B in range(NBLK):
        nz = min(ZBLK, NELEM - B * ZBLK)
        c6t[: nz * NREF, B * NY:(B + 1) * NY] = (
            tr[B * ZBLK:B * ZBLK + nz].reshape(nz * NREF, NY))
    c6t16 = c6t.astype(BF)

    cores = []
    for k, (a0, a1) in enumerate(atom_ranges):
        nloc = a1 - a0
        degs = counts[a0:a1]
        li = np.searchsorted(LS, degs, side="left")
        part = np.full(nloc, -1, np.int64)
        acol_of = np.full(nloc, -1, np.int64)
        scolb = np.full(nloc, -1, np.int64)
        agrid = np.full((P, ACOLS), -1, np.int64)
        for bi, (L, n, scol0, acol0) in enumerate(groups):
            sel = np.nonzero((li == bi) & (degs > 0))[0]
            if len(sel) == 0:
                continue
            t = np.arange(len(sel))
            c = t // P
            p = t % P
            part[sel] = p
            acol_of[sel] = acol0 + c
            scolb[sel] = scol0 + c * L
            agrid[p, acol0 + c] = sel + a0

        e0, e1 = ccum[a0], ccum[a1]
        eo = order[e0:e1]
        il = i_s[e0:e1] - a0
        pp = part[il]
        cc = scolb[il] + pos[e0:e1]

        s1 = np.zeros((P, 5, COLS), BF)
        s1[:, 3, :] = BF(-1e4)
        s1[pp, 0, cc] = dr_vec[eo, 0].astype(BF)
        s1[pp, 1, cc] = dr_vec[eo, 1].astype(BF)
        s1[pp, 2, cc] = dr_vec[eo, 2].astype(BF)
        s1[pp, 3, cc] = rcov_a[j[eo]].astype(BF)
        s1[pp, 4, cc] = r4r2_a[j[eo]].astype(BF)

        am = agrid >= 0
        atrc = np.zeros((P, ACOLS), np.float32)
        atr4 = np.zeros((P, ACOLS), np.float32)
        atrc[am] = rcov_a[agrid[am]]
        atr4[am] = r4r2_a[agrid[am]]
        atref = np.zeros((P, 5, ACOLS), np.float32)
        pr, cr = np.nonzero(am)
        atref[pr, :, cr] = refcn_a[agrid[pr, cr]]

        # element grid
        eo_at = eorders[k]
        egrid = np.full((P, C), -1, np.int64)
        Rt = np.full((P, C, 5 * ZBLK), -1e4, np.float32)
        zb = Zi_all[eo_at] // ZBLK
        for B in range(NBLK):
            sel = eo_at[zb == B]
            t = np.arange(len(sel))
            col = CBoff[B] + t // P
            p = t % P
            egrid[p, col] = sel
            z = Zi_all[sel]
            zl = z - B * ZBLK
            Rt[p, col, :] = -1e4
            for r in range(NREF):
                Rt[p, col, 5 * zl + r] = ref_cn_table[z, r]
        Rt16 = Rt.astype(BF)

        cores.append(dict(
            s1=s1, atrc=atrc, atr4=atr4, atref=atref, agrid=agrid,
            pp=pp, cc=cc, jglob=j[eo], ziedge=Zi_all[i_s[e0:e1]],
            egrid=egrid, Rt=Rt16,
        ))

    return dict(pieces=pieces, COLS=COLS, ACOLS=ACOLS, C=C,
                blk_of_col=blk_of_col, c6t=c6t16, cores=cores, N=N, E=E)


def _new_nc():
    return bacc.Bacc("TRN2", target_bir_lowering=False, debug=False,
                     num_devices=NCORES)


def _build_l1a(pieces, COLS, ACOLS):
    nc = _new_nc()
    s1 = nc.declare_dram_parameter("s1", [P, 5 * COLS], BF16, isOutput=False)
    atrc = nc.declare_dram_parameter("atrc", [P, ACOLS], F32, isOutput=False)
    atr4 = nc.declare_dram_parameter("atr4", [P, ACOLS], F32, isOutput=False)
    atref = nc.declare_dram_parameter("atref", [P, 5 * ACOLS], F32, isOutput=False)
    cno = nc.declare_dram_parameter("cn", [P, ACOLS], F32, isOutput=True)
    w5po = nc.declare_dram_parameter("w5p", [P, 5 * ACOLS], F32, isOutput=True)
    drawo = nc.declare_dram_parameter("draw", [P, COLS], BF16, isOutput=True)
    s1v = s1[:].rearrange("p (f c) -> p f c", f=5)

    with ExitStack() as ctx, nc.allow_low_precision("stat noise averages out"):
        tc = ctx.enter_context(tile.TileContext(nc))
        persist = ctx.enter_context(tc.tile_pool(name="persist", bufs=1))
        spool = ctx.enter_context(tc.tile_pool(name="stream", bufs=2))
        wpool = ctx.enter_context(tc.tile_pool(name="work", bufs=2))

        atrc_t = persist.tile([P, ACOLS], F32)
        nc.sync.dma_start(atrc_t[:], atrc[:])
        atr4_t = persist.tile([P, ACOLS], F32)
        nc.sync.dma_start(atr4_t[:], atr4[:])
        atref_t = persist.tile([P, 5 * ACOLS], F32)
        nc.sync.dma_start(atref_t[:], atref[:])
        atrcb = persist.tile([P, ACOLS], BF16)
        nc.scalar.activation(atrcb[:], atrc_t[:], AF.Copy)
        atr43b = persist.tile([P, ACOLS], BF16)
        nc.scalar.activation(atr43b[:], atr4_t[:], AF.Copy, scale=3.0)
        b_negk = persist.tile([P, 1], F32)
        nc.vector.memset(b_negk[:], -KCN)
        b_a2 = persist.tile([P, 1], F32)
        nc.vector.memset(b_a2[:], A2)
        cn_t = persist.tile([P, ACOLS], F32)

        for _rep in range(REPEAT):
          for (L, n_p, scol, acol) in pieces:
            W = n_p * L
            st = spool.tile([P, 5 * W], BF16, tag="st")
            nc.sync.dma_start(
                st[:].rearrange("p (f w) -> p f w", f=5),
                s1v[:, :, scol:scol + W])
            v = st[:].rearrange("p (f w) -> p f w", f=5)
            px, py, pz, prj, pqj = (v[:, q, :] for q in range(5))

            def wt(tag):
                return wpool.tile([P, W], BF16, tag=tag, name=tag)

            bx = wt("bx"); by = wt("by"); bz = wt("bz")
            nc.scalar.activation(bx[:], px, AF.Square)
            nc.scalar.activation(by[:], py, AF.Square)
            nc.vector.tensor_tensor(bz[:], pz, pz, ALU.mult)
            s2t = wt("s2t")
            nc.vector.tensor_tensor(s2t[:], bx[:], by[:], ALU.add)
            sful = wt("sful")
            nc.vector.tensor_tensor(sful[:], s2t[:], bz[:], ALU.add)

            # --- coordination number
            dr = wt("dr")
            nc.scalar.activation(dr[:], sful[:], AF.Sqrt, scale=IB2)
            rdr = wt("rdr")
            nc.vector.reciprocal(rdr[:], dr[:])
            rc = wt("rc")
            rci = atrcb[:, acol:acol + n_p].unsqueeze(-1).to_broadcast([P, n_p, L])
            nc.vector.tensor_tensor(
                rc[:].rearrange("p (a l) -> p a l", a=n_p),
                prj.rearrange("p (a l) -> p a l", a=n_p), rci, ALU.add)
            targ = wt("targ")
            nc.vector.tensor_tensor(targ[:], rc[:], rdr[:], ALU.mult)
            cnt = wt("cnt")
            nc.scalar.activation(cnt[:], targ[:], AF.Sigmoid, scale=KCN,
                                 bias=b_negk[:])
            nc.vector.tensor_reduce(
                cn_t[:, acol:acol + n_p],
                cnt[:].rearrange("p (a l) -> p a l", a=n_p), AX.X, ALU.add)

            # --- BJ damping Draw = S6/S8 * i6 + qq * i8
            qq = wt("qq")
            r4i = atr43b[:, acol:acol + n_p].unsqueeze(-1).to_broadcast([P, n_p, L])
            nc.vector.tensor_tensor(
                qq[:].rearrange("p (a l) -> p a l", a=n_p),
                pqj.rearrange("p (a l) -> p a l", a=n_p), r4i, ALU.mult)
            rrs = wt("rrs")
            nc.scalar.activation(rrs[:], qq[:], AF.Sqrt, scale=A1 * A1)
            rr2 = wt("rr2")
            nc.scalar.activation(rr2[:], rrs[:], AF.Square, bias=b_a2[:])
            t2 = wt("t2")
            nc.scalar.activation(t2[:], rr2[:], AF.Square)
            rr6 = wt("rr6")
            nc.vector.tensor_tensor(rr6[:], t2[:], rr2[:], ALU.mult)
            rr8 = wt("rr8")
            nc.gpsimd.tensor_tensor(rr8[:], rr6[:], rr2[:], ALU.mult)
            t3 = wt("t3")
            nc.scalar.activation(t3[:], sful[:], AF.Square, scale=IB2)
            dr6 = wt("dr6")
            nc.vector.scalar_tensor_tensor(dr6[:], t3[:], IB2, sful[:],
                                           ALU.mult, ALU.mult)
            den6 = wt("den6")
            nc.vector.tensor_tensor(den6[:], dr6[:], rr6[:], ALU.add)
            i6 = wt("i6")
            nc.vector.reciprocal(i6[:], den6[:])
            dr8 = wt("dr8")
            nc.vector.scalar_tensor_tensor(dr8[:], sful[:], IB2, dr6[:],
                                           ALU.mult, ALU.mult)
            den8 = wt("den8")
            nc.gpsimd.tensor_tensor(den8[:], dr8[:], rr8[:], ALU.add)
            i8 = wt("i8")
            nc.vector.reciprocal(i8[:], den8[:])
            t8 = wt("t8")
            nc.vector.tensor_tensor(t8[:], qq[:], i8[:], ALU.mult)
            drawt = wt("drawt")
            nc.vector.scalar_tensor_tensor(drawt[:], i6[:], S6 / S8, t8[:],
                                           ALU.mult, ALU.add)
            nc.sync.dma_start(drawo[:, scol:scol + W], drawt[:])

        # --- per-atom tail: w5p = -HA*S8/2 * w / (sum w + eps)
        atref_v = atref_t[:].rearrange("p (f a) -> p f a", f=5)
        dcn = persist.tile([P, 5 * ACOLS], F32)
        dcn_v = dcn[:].rearrange("p (f a) -> p f a", f=5)
        nc.vector.tensor_tensor(
            dcn_v, atref_v,
            cn_t[:].unsqueeze(1).to_broadcast([P, 5, ACOLS]), ALU.subtract)
        nc.scalar.activation(dcn[:], dcn[:], AF.Square)
        w5p_t = persist.tile([P, 5 * ACOLS], F32)
        nc.scalar.activation(w5p_t[:], dcn[:], AF.Exp, scale=-WF)
        w5p_v = w5p_t[:].rearrange("p (f a) -> p f a", f=5)
        wsum = persist.tile([P, ACOLS], F32)
        nc.vector.tensor_tensor(wsum[:], w5p_v[:, 0, :], w5p_v[:, 1, :], ALU.add)
        nc.vector.tensor_tensor(wsum[:], wsum[:], w5p_v[:, 2, :], ALU.add)
        nc.vector.tensor_tensor(wsum[:], wsum[:], w5p_v[:, 3, :], ALU.add)
        nc.vector.tensor_tensor(wsum[:], wsum[:], w5p_v[:, 4, :], ALU.add)
        nc.vector.tensor_scalar_add(wsum[:], wsum[:], EPS32)
        winv = persist.tile([P, ACOLS], F32)
        nc.vector.reciprocal(winv[:], wsum[:])
        nc.vector.tensor_scalar_mul(winv[:], winv[:], -HA * S8 / 2.0)
        nc.vector.tensor_tensor(
            w5p_v, w5p_v, winv[:].unsqueeze(1).to_broadcast([P, 5, ACOLS]),
            ALU.mult)
        nc.sync.dma_start(w5po[:], w5p_t[:])
        nc.sync.dma_start(cno[:], cn_t[:])
    nc.compile()
    return nc


def _build_l1b(C, blk_of_col):
    nc = _new_nc()
    K = 5 * ZBLK
    rt_p = nc.declare_dram_parameter("rt", [P, C * K], BF16, isOutput=False)
    cn_p = nc.declare_dram_parameter("cne", [P, C], F32, isOutput=False)
    c6_p = nc.declare_dram_parameter("c6t", [K, NBLK * NY], BF16, isOutput=False)
    y_p = nc.declare_dram_parameter("y", [P, C * NY], BF16, isOutput=True)

    with ExitStack() as ctx, nc.allow_low_precision("stat noise averages out"):
        tc = ctx.enter_context(tile.TileContext(nc))
        persist = ctx.enter_context(tc.tile_pool(name="persist", bufs=1))
        wp = ctx.enter_context(tc.tile_pool(name="work", bufs=3))
        yp = ctx.enter_context(tc.tile_pool(name="ywork", bufs=3))
        pp_t = ctx.enter_context(tc.tile_pool(name="ps_t", bufs=2, space="PSUM"))
        pp_y = ctx.enter_context(tc.tile_pool(name="ps_y", bufs=4, space="PSUM"))

        ident = persist.tile([P, P], BF16)
        masks.make_identity(nc, ident[:])
        rt_t = persist.tile([P, C * K], BF16)
        nc.sync.dma_start(rt_t[:], rt_p[:])
        rt_v = rt_t[:].rearrange("p (c k) -> p c k", c=C)
        cn_t = persist.tile([P, C], F32)
        nc.sync.dma_start(cn_t[:], cn_p[:])
        c6_t = persist.tile([K, NBLK * NY], BF16)
        nc.sync.dma_start(c6_t[:], c6_p[:])
        negcn = persist.tile([P, C], F32)
        nc.vector.tensor_scalar_mul(negcn[:], cn_t[:], -1.0)

        for _rep in range(REPEAT_B):
          for c in range(C):
            B = int(blk_of_col[c])
            t1 = wp.tile([P, K], BF16, tag="t1")
            nc.scalar.activation(t1[:], rt_v[:, c, :], AF.Square,
                                 bias=negcn[:, c:c + 1])
            wsp = wp.tile([P, K], BF16, tag="wsp")
            nc.scalar.activation(wsp[:], t1[:], AF.Exp, scale=-WF)
            ws = wp.tile([P, 1], F32, tag="ws")
            nc.vector.tensor_reduce(ws[:], wsp[:], AX.X, ALU.add)
            nc.vector.tensor_scalar_add(ws[:], ws[:], EPS32)
            wi = wp.tile([P, 1], F32, tag="wi")
            nc.vector.reciprocal(wi[:], ws[:])

            pst = pp_t.tile([P, P], BF16, tag="pst")
            nc.tensor.transpose(pst[:K, :], wsp[:], ident[:])
            wT = wp.tile([P, P], BF16, tag="wT")
            nc.vector.tensor_copy(wT[:K, :], pst[:K, :])

            yps = pp_y.tile([P, NY], F32, tag="yps")
            nc.tensor.matmul(yps[:], wT[:K, :], c6_t[:, B * NY:(B + 1) * NY],
                             start=True, stop=True)
            ysb = yp.tile([P, NY], BF16, tag="ysb")
            if c % 3 == 2:
                nc.vector.tensor_scalar(ysb[:], yps[:], wi[:], None, ALU.mult)
            else:
                nc.scalar.activation(ysb[:], yps[:], AF.Copy, scale=wi[:])
            nc.sync.dma_start(y_p[:, c * NY:(c + 1) * NY], ysb[:])
    nc.compile()
    return nc


def _build_l2(pieces, COLS, ACOLS):
    nc = _new_nc()
    draw = nc.declare_dram_parameter("draw", [P, COLS], BF16, isOutput=False)
    y5 = nc.declare_dram_parameter("y5", [P, 5 * COLS], BF16, isOutput=False)
    w5p = nc.declare_dram_parameter("w5p", [P, 5 * ACOLS], F32, isOutput=False)
    eto = nc.declare_dram_parameter("etot", [1, 1], F32, isOutput=True)
    y5v = y5[:].rearrange("p (s c) -> p s c", s=5)

    with ExitStack() as ctx, nc.allow_low_precision("stat noise averages out"):
        tc = ctx.enter_context(tile.TileContext(nc))
        persist = ctx.enter_context(tc.tile_pool(name="persist", bufs=1))
        spool = ctx.enter_context(tc.tile_pool(name="stream", bufs=2))
        wpool = ctx.enter_context(tc.tile_pool(name="work", bufs=2))
        ppool = ctx.enter_context(tc.tile_pool(name="psum", bufs=1, space="PSUM"))

        w5p_t = persist.tile([P, 5 * ACOLS], F32)
        nc.sync.dma_start(w5p_t[:], w5p[:])
        w5p_v = w5p_t[:].rearrange("p (s a) -> p s a", s=5)
        eacc = persist.tile([P, 1], F32)
        nc.vector.memset(eacc[:], 0.0)

        for _rep in range(REPEAT):
          for (L, n_p, scol, acol) in pieces:
            W = n_p * L
            dt_ = spool.tile([P, W], BF16, tag="dt")
            nc.sync.dma_start(dt_[:], draw[:, scol:scol + W])
            yt = spool.tile([P, 5 * W], BF16, tag="yt")
            nc.sync.dma_start(
                yt[:].rearrange("p (s w) -> p s w", s=5),
                y5v[:, :, scol:scol + W])
            t = wpool.tile([P, 5 * W], BF16, tag="t")
            nc.vector.tensor_tensor(
                t[:].rearrange("p (s w) -> p s w", s=5),
                yt[:].rearrange("p (s w) -> p s w", s=5),
                dt_[:].unsqueeze(1).to_broadcast([P, 5, W]), ALU.mult)
            R5 = wpool.tile([P, 5, n_p], BF16, tag="R5")
            nc.vector.tensor_reduce(
                R5[:], t[:].rearrange("p (s a l) -> p s a l", s=5, a=n_p),
                AX.X, ALU.add)
            junk = wpool.tile([P, 5, n_p], F32, tag="junk")
            ep = wpool.tile([P, 1], F32, tag="ep")
            nc.vector.scalar_tensor_tensor(
                junk[:], R5[:], 0.0, w5p_v[:, :, acol:acol + n_p],
                ALU.add, ALU.mult, accum_out=ep[:])
            nc.vector.tensor_tensor(eacc[:], eacc[:], ep[:], ALU.add)

        ones = persist.tile([P, 1], F32)
        nc.vector.memset(ones[:], 1.0)
        ps = ppool.tile([1, 1], F32)
        nc.tensor.matmul(ps[:], ones[:], eacc[:], start=True, stop=True)
        esb = persist.tile([1, 1], F32)
        nc.scalar.copy(esb[:], ps[:])
        nc.sync.dma_start(eto[:], esb[:])
    nc.compile()
    return nc


def _get_kernels(prep):
    key = (tuple(prep["pieces"]), prep["COLS"], prep["ACOLS"], prep["C"],
           tuple(prep["blk_of_col"].tolist()), REPEAT, REPEAT_B)
    if key not in _cache:
        _cache[key] = (
            _build_l1a(prep["pieces"], prep["COLS"], prep["ACOLS"]),
            _build_l1b(prep["C"], prep["blk_of_col"]),
            _build_l2(prep["pieces"], prep["COLS"], prep["ACOLS"]),
        )
    return _cache[key]


def _in1(prep):
    return [{"s1": c["s1"].reshape(P, -1), "atrc": c["atrc"],
             "atr4": c["atr4"], "atref": c["atref"].reshape(P, -1)}
            for c in prep["cores"]]


def _join_cn(prep, r1results):
    cn_full = np.zeros(prep["N"], np.float32)
    for k, c in enumerate(prep["cores"]):
        m = c["agrid"] >= 0
        cn_full[c["agrid"][m]] = r1results[k]["cn"][m]
    return cn_full


def _in1b(prep, cn_full):
    ins = []
    for c in prep["cores"]:
        cnE = np.zeros((P, prep["C"]), np.float32)
        m = c["egrid"] >= 0
        cnE[m] = cn_full[c["egrid"][m]]
        ins.append({"rt": c["Rt"].reshape(P, -1), "cne": cnE,
                    "c6t": prep["c6t"]})
    return ins


def _join_y(prep, rbresults):
    N, C = prep["N"], prep["C"]
    yfull = np.zeros((N, NELEM, NREF), BF)
    for k, c in enumerate(prep["cores"]):
        yk = rbresults[k]["y"].reshape(P, C, NELEM, NREF)
        m = c["egrid"] >= 0
        yfull[c["egrid"][m]] = yk[m]
    ins = []
    for k, c in enumerate(prep["cores"]):
        ye = yfull[c["jglob"], c["ziedge"]]  # [Ecore, 5] bf16
        s2y = np.zeros((P, 5, prep["COLS"]), BF)
        for s in range(5):
            s2y[c["pp"], s, c["cc"]] = ye[:, s]
        ins.append({"y5": s2y.reshape(P, -1)})
    return ins


def kernel(dr_vec, ref_cn_table, ref_c6_table, r4r2_table, rcov_table, numbers, idx):
    # smooth_cutoff(dr, 20, 25) and (55, 60) are identically 1 for this data
    assert np.sqrt((dr_vec.astype(np.float64) ** 2).sum(-1)).max() / BOHR < 19.0
    prep = _prep(dr_vec, ref_cn_table, ref_c6_table, r4r2_table, rcov_table,
                 numbers, idx)
    nc1a, nc1b, nc2 = _get_kernels(prep)

    r1 = run_bass_kernel_spmd(nc1a, _in1(prep), list(range(NCORES)))
    cn_full = _join_cn(prep, r1.results)
    rb = run_bass_kernel_spmd(nc1b, _in1b(prep, cn_full), list(range(NCORES)))
    iny = _join_y(prep, rb.results)
    in2 = [{"draw": r1.results[k]["draw"], "w5p": r1.results[k]["w5p"],
            **iny[k]} for k in range(NCORES)]
    r2 = run_bass_kernel_spmd(nc2, in2, list(range(NCORES)))

    parts = [r2.results[k]["etot"].reshape(()) for k in range(NCORES)]
    return np.float32(np.sum(np.stack(parts)))
